# revision 15
# baseline (speedup 1.0000x reference)
"""Trainium2 Bass kernel for nn_KCLWONegLoss.

Reference math (all f32):
    sums    = embs.sum(axis=1)                          # [64, 512]
    pos[p]  = cos(sums[p], sums[p+8])                   # p in 0..55
    a       = g1[neg1]; b = g2[neg2]                    # [56, 32, 512]
    sim[p,d]= cos over K axis (32) of a[p,:,d], b[p,:,d]
    num     = exp(pos/0.1)
    den     = num + sum_d exp(sim/0.1)
    loss    = 2 * sum_p (log(den) - pos/0.1)

Sharding: data-parallel over the D=64 group axis (8 groups/core) for the
embs reduction; the 56 positive pairs are sharded 7/core, with each core
receiving only its gathered rows of g1/g2 (row-gather done host-side at
shard-build time). Per-core device outputs: a [7, 513] tile (group sums
0..6 + the 7 per-pair negative-denominator sums in col 512) and a
[1, 512] tile (group sum 7). The final 56 cosines + log-sum (~0.1 Mflop)
are assembled on host in float64.

Device schedule (per core, HBM-stream-bound at ~358 GB/s):
  - all big input DMAs are contiguous-per-partition and issued on the SP
    HWDGE queue in consumption order (gather tiles first, then the embs
    group chunks; the last group is split into two d-halves so its
    pre-reduction can overlap the final quarter of the stream); consts go
    on the ACT HWDGE queue in parallel.
  - embs group chunk [256,512] is loaded as [128, 2, 512] (partition p =
    rows 2p, 2p+1); the 256->128 pre-reduction is a DVE tensor_add of the
    two halves (TT has 2 read ports: ~2x faster than the strided
    TENSOR_REDUCE). Groups 0..6 accumulate via 8-col selector matmuls
    into one PSUM chain that stops at g6, so rows 0..6 are copied to SBUF
    while group 7 (own single-column chain -> [1,512] PSUM, partition 0)
    is still in flight. Only the tiny second output DMA's completion
    receipt sits on the critical tail.
  - negative path: 6 elementwise muls on the otherwise-idle GpSimd (keeps
    DVE free to chase the stream) -> 6 block-selector matmuls -> sim =
    dot * rsqrt(asq) * rsqrt(bsq) -> Exp with accum_out writing the den
    column straight into the output tile.
  - gather rows (7 pairs * 32 = 224 per side) are loaded without the pad
    bytes (full [128,2,512] a0/b0 + [96,2,512] a1/b1; the 32 fake pad
    rows are memset to 1.0 on-chip so the fake 8th pair stays finite; its
    den lands in out row 7's den slot, which is never read back).
"""

import numpy as np

D, NG, DIM = 64, 256, 512
L, K = 8, 32
P = D - L               # 56 positive pairs
TEMP = 0.1
EPS = 1e-8
N_CORES = 8
GPC = D // N_CORES      # 8 groups per core
PPC = P // N_CORES      # 7 pairs per core
ROWS = PPC * K          # 224 gathered rows per core per side
HD = DIM // 2

_PROGRAM = None         # cached compiled Bass program
LAST_RESULTS = None     # BassKernelResults of the most recent run (for test.py)


def _build_program():
    import concourse.bass as bass
    import concourse.tile as tile
    from concourse import bacc, mybir

    f32 = mybir.dt.float32
    f32r = mybir.dt.float32r
    AF = mybir.ActivationFunctionType
    nc = bacc.Bacc("TRN2", target_bir_lowering=False, debug=False)

    embs_t = nc.dram_tensor("embs_s", [GPC, NG, DIM], f32, kind="ExternalInput")
    gabA_t = nc.dram_tensor("gabA", [128, 2, DIM], f32, kind="ExternalInput")
    gabB_t = nc.dram_tensor("gabB", [96, 2, DIM], f32, kind="ExternalInput")
    consts_t = nc.dram_tensor("consts", [128, 81], f32, kind="ExternalInput")
    out_t = nc.dram_tensor("out", [PPC, DIM + 1], f32, kind="ExternalOutput")
    out7_t = nc.dram_tensor("out7", [1, DIM], f32, kind="ExternalOutput")

    with tile.TileContext(nc) as tc:
        with (
            tc.tile_pool(name="pool", bufs=1) as pool,
            tc.tile_pool(name="psum", bufs=1, space=bass.MemorySpace.PSUM) as psum,
        ):
            # gather tile: cols (a0, b0, a1, b1); pad = partitions 96..127
            # of (a1, b1), memset to 1.0 (fake 8th pair -> asq=bsq=K, finite)
            gab = pool.tile([128, 4, DIM], f32, tag="gab")
            nc.gpsimd.memset(gab[96:128, 2:4, :], 1.0)

            # consts on the ACT HWDGE queue (parallel to the SP stream):
            #   cols 8g..8g+8: selector S_g (ones col 8g+g -> output row g)
            #   cols 64..71  : neg t0 block-ones (pair m -> rows 32m..32m+32)
            #   cols 72..79  : neg t1 block-ones (pairs 4..6 + pad block col 79)
            #   col 80       : all-ones (group-7 single-column selector)
            consts = pool.tile([128, 81], f32r, tag="consts")
            nc.scalar.dma_start(consts[:], consts_t.ap().bitcast(f32r))

            # SP stream, consumption order: gather rows then embs chunks;
            # the last chunk split into d-halves
            nc.sync.dma_start(gab[:, 0:2, :], gabA_t.ap())
            nc.sync.dma_start(gab[0:96, 2:4, :], gabB_t.ap())
            etiles = []
            for g in range(GPC):
                e = pool.tile([128, 2, DIM], f32r, tag=f"e{g}")
                src = embs_t.ap()[g].rearrange("(p h) d -> p h d", h=2).bitcast(f32r)
                if g < GPC - 1:
                    nc.sync.dma_start(e[:], src)
                else:
                    nc.sync.dma_start(e[:, :, 0:HD], src[:, :, 0:HD])
                    nc.sync.dma_start(e[:, :, HD:DIM], src[:, :, HD:DIM])
                etiles.append(e)

            # --- negative path: elementwise products on GpSimd ---
            prods = []
            with nc.allow_low_precision(reason="f32r is fp32-width"):
                for t in range(2):
                    a, b = gab[:, 2 * t, :], gab[:, 2 * t + 1, :]
                    pr = pool.tile([128, DIM], f32r, tag=f"prod{t}")
                    aa = pool.tile([128, DIM], f32r, tag=f"aa{t}")
                    bb = pool.tile([128, DIM], f32r, tag=f"bb{t}")
                    nc.gpsimd.tensor_mul(pr[:], a, b)
                    nc.gpsimd.tensor_mul(aa[:], a, a)
                    nc.gpsimd.tensor_mul(bb[:], b, b)
                    prods.append((pr, aa, bb))

            # block-selector matmuls: K-sums per pair -> [8, 512] PSUM
            dot_ps = psum.tile([8, DIM], f32, tag="dot")
            asq_ps = psum.tile([8, DIM], f32, tag="asq")
            bsq_ps = psum.tile([8, DIM], f32, tag="bsq")
            for t, (pr, aa, bb) in enumerate(prods):
                blk = consts[:, 64 + 8 * t:72 + 8 * t]
                st, sp = (t == 0), (t == 1)
                nc.tensor.matmul(dot_ps[:], blk, pr[:], start=st, stop=sp)
                nc.tensor.matmul(asq_ps[:], blk, aa[:], start=st, stop=sp)
                nc.tensor.matmul(bsq_ps[:], blk, bb[:], start=st, stop=sp)

            # --- group sums: h-add on DVE, selector matmul accumulation.
            # Groups 0..6 -> one [8,512] chain (stop at g6); group 7 gets a
            # single-column chain into [1,512] (partition 0) so its copy
            # and output DMA stay legal and tiny on the tail.
            sums_ps = psum.tile([8, DIM], f32, tag="sums")
            s7_ps = psum.tile([1, DIM], f32, tag="s7")
            ctiles = [
                pool.tile([128, DIM], f32r, tag=f"c{g}", name=f"c{g}")
                for g in range(GPC)
            ]

            out_sb = pool.tile([GPC, DIM + 1], f32, tag="out_sb")
            out7_sb = pool.tile([1, DIM], f32, tag="out7_sb")
            dot_sb = pool.tile([8, DIM], f32, tag="dot_sb")
            ai = pool.tile([8, DIM], f32, tag="ai")
            bi = pool.tile([8, DIM], f32, tag="bi")
            tmp = pool.tile([8, DIM], f32, tag="tmp")
            sim = pool.tile([8, DIM], f32, tag="sim")
            etile = pool.tile([8, DIM], f32, tag="etile")

            def hadd(g):
                with nc.allow_low_precision(reason="f32r is fp32-width"):
                    nc.vector.tensor_add(
                        ctiles[g][:], etiles[g][:, 0, :], etiles[g][:, 1, :]
                    )

            def selmm(g):
                nc.tensor.matmul(
                    sums_ps[:],
                    consts[:, 8 * g:8 * g + 8],
                    ctiles[g][:],
                    start=(g == 0),
                    stop=(g == GPC - 2),
                )

            # sim = dot * rsqrt(asq) * rsqrt(bsq)  (cosine; the reference eps
            # guard can never bind for randn inputs). The whole negative-path
            # epilogue runs on ACT + GpSimd: the DVE stream stays pure
            # stream-gated h-adds so the scheduler can't stall it on the
            # (late) ai/bi products.
            nc.scalar.copy(dot_sb[:], dot_ps[:])
            nc.scalar.activation(ai[:], asq_ps[:], AF.Abs_reciprocal_sqrt)
            nc.scalar.activation(bi[:], bsq_ps[:], AF.Abs_reciprocal_sqrt)
            nc.gpsimd.tensor_mul(tmp[:], dot_sb[:], ai[:])
            nc.gpsimd.tensor_mul(sim[:], tmp[:], bi[:])
            # exp(sim/T), row-sum fused straight into the out tile's den col
            nc.scalar.activation(
                etile[:], sim[:], AF.Exp,
                scale=float(1.0 / TEMP), accum_out=out_sb[:, DIM:DIM + 1],
            )
            for g in range(GPC - 1):
                hadd(g)
                selmm(g)
            nc.scalar.copy(out_sb[0:PPC, 0:DIM], sums_ps[0:PPC, :])
            nc.sync.dma_start(out_t.ap(), out_sb[0:PPC, :])

            g = GPC - 1
            with nc.allow_low_precision(reason="f32r is fp32-width"):
                nc.vector.tensor_add(
                    ctiles[g][:, 0:HD],
                    etiles[g][:, 0, 0:HD], etiles[g][:, 1, 0:HD],
                )
                nc.vector.tensor_add(
                    ctiles[g][:, HD:DIM],
                    etiles[g][:, 0, HD:DIM], etiles[g][:, 1, HD:DIM],
                )
            nc.tensor.matmul(
                s7_ps[:], consts[:, 80:81], ctiles[g][:], start=True, stop=True
            )
            nc.scalar.copy(out7_sb[:], s7_ps[:])
            nc.sync.dma_start(out7_t.ap(), out7_sb[:])

    nc.compile()
    return nc


def _get_program():
    global _PROGRAM
    if _PROGRAM is None:
        _PROGRAM = _build_program()
    return _PROGRAM


def kernel(embs, g0, g1, g2, neg1, neg2, **_unused):
    global LAST_RESULTS
    from concourse.bass_utils import run_bass_kernel_spmd

    embs = np.ascontiguousarray(np.asarray(embs, dtype=np.float32))
    g1 = np.ascontiguousarray(np.asarray(g1, dtype=np.float32))
    g2 = np.ascontiguousarray(np.asarray(g2, dtype=np.float32))
    neg1 = np.asarray(neg1).astype(np.int64)
    neg2 = np.asarray(neg2).astype(np.int64)

    consts = np.zeros((128, 81), np.float32)
    for g in range(GPC):
        consts[:, 8 * g + g] = 1.0                      # selector S_g, column g
    for m in range(4):
        consts[m * 32:(m + 1) * 32, 64 + m] = 1.0       # t0 pair blocks
    for j in range(3):
        consts[j * 32:(j + 1) * 32, 72 + 4 + j] = 1.0   # t1 pair blocks 4..6
    consts[96:128, 79] = 1.0                            # fake pad pair
    consts[:, 80] = 1.0                                 # group-7 ones column

    in_maps = []
    for c in range(N_CORES):
        idx1 = neg1[c * PPC:(c + 1) * PPC].reshape(-1)
        idx2 = neg2[c * PPC:(c + 1) * PPC].reshape(-1)
        gabA = np.empty((128, 2, DIM), np.float32)
        gabA[:, 0, :] = g1[idx1[:128]]
        gabA[:, 1, :] = g2[idx2[:128]]
        gabB = np.empty((96, 2, DIM), np.float32)
        gabB[:, 0, :] = g1[idx1[128:]]
        gabB[:, 1, :] = g2[idx2[128:]]
        in_maps.append({
            "embs_s": embs[c * GPC:(c + 1) * GPC],
            "gabA": gabA,
            "gabB": gabB,
            "consts": consts,
        })

    nc = _get_program()
    res = run_bass_kernel_spmd(nc, in_maps, core_ids=list(range(N_CORES)))
    LAST_RESULTS = res

    sums = np.empty((D, DIM), np.float64)
    den_neg = np.empty((P,), np.float64)
    for c in range(N_CORES):
        o = res.results[c]["out"]
        sums[c * GPC:c * GPC + PPC] = o[:, :DIM]
        sums[c * GPC + GPC - 1] = res.results[c]["out7"][0]
        den_neg[c * PPC:(c + 1) * PPC] = o[:, DIM]

    s_i, s_j = sums[:P], sums[L:]
    na = np.maximum(np.sqrt((s_i * s_i).sum(1)), EPS)
    nb = np.maximum(np.sqrt((s_j * s_j).sum(1)), EPS)
    pos = (s_i * s_j).sum(1) / (na * nb)
    num = np.exp(pos / TEMP)
    den = num + den_neg
    total = 2.0 * np.sum(np.log(den) - pos / TEMP)
    return np.asarray(total, dtype=np.float32)


# revision 18
# speedup vs baseline: 1.0059x; 1.0059x over previous
"""Trainium2 Bass kernel for nn_KCLWONegLoss.

Reference math (all f32):
    sums    = embs.sum(axis=1)                          # [64, 512]
    pos[p]  = cos(sums[p], sums[p+8])                   # p in 0..55
    a       = g1[neg1]; b = g2[neg2]                    # [56, 32, 512]
    sim[p,d]= cos over K axis (32) of a[p,:,d], b[p,:,d]
    num     = exp(pos/0.1)
    den     = num + sum_d exp(sim/0.1)
    loss    = 2 * sum_p (log(den) - pos/0.1)

Sharding: data-parallel over the D=64 group axis (8 groups/core) for the
embs reduction; the 56 positive pairs are sharded 7/core, with each core
receiving only its gathered rows of g1/g2 (row-gather done host-side at
shard-build time). Per-core device outputs: a [7, 513] tile (group sums
0..6 + the 7 per-pair negative-denominator sums in col 512) and a
[1, 512] tile (group sum 7). The final 56 cosines + log-sum (~0.1 Mflop)
are assembled on host in float64.

Device schedule (per core, HBM-stream-bound at ~358 GB/s):
  - all big input DMAs are contiguous-per-partition and issued on the SP
    HWDGE queue in consumption order (gather tiles first, then the embs
    group chunks; the last group is split into two d-halves so its
    pre-reduction can overlap the final quarter of the stream); consts go
    on the ACT HWDGE queue in parallel.
  - embs group chunk [256,512] is loaded as [128, 2, 512] (partition p =
    rows 2p, 2p+1); the 256->128 pre-reduction is a DVE tensor_add of the
    two halves (TT has 2 read ports: ~2x faster than the strided
    TENSOR_REDUCE). Groups 0..6 accumulate via 8-col selector matmuls
    into one PSUM chain that stops at g6, so rows 0..6 are copied to SBUF
    while group 7 (own single-column chain -> [1,512] PSUM, partition 0)
    is still in flight. Only the tiny second output DMA's completion
    receipt sits on the critical tail.
  - negative path: 6 elementwise muls on the otherwise-idle GpSimd (keeps
    DVE free to chase the stream) -> 6 block-selector matmuls -> sim =
    dot * rsqrt(asq) * rsqrt(bsq) -> Exp with accum_out writing the den
    column straight into the output tile.
  - gather rows (7 pairs * 32 = 224 per side) are loaded without the pad
    bytes (full [128,2,512] a0/b0 + [96,2,512] a1/b1; the 32 fake pad
    rows are memset to 1.0 on-chip so the fake 8th pair stays finite; its
    den lands in out row 7's den slot, which is never read back).
"""

import numpy as np

D, NG, DIM = 64, 256, 512
L, K = 8, 32
P = D - L               # 56 positive pairs
TEMP = 0.1
EPS = 1e-8
N_CORES = 8
GPC = D // N_CORES      # 8 groups per core
PPC = P // N_CORES      # 7 pairs per core
ROWS = PPC * K          # 224 gathered rows per core per side
HD = DIM // 2

_PROGRAM = None         # cached compiled Bass program
LAST_RESULTS = None     # BassKernelResults of the most recent run (for test.py)


def _build_program():
    import concourse.bass as bass
    import concourse.tile as tile
    from concourse import bacc, mybir

    f32 = mybir.dt.float32
    f32r = mybir.dt.float32r
    AF = mybir.ActivationFunctionType
    nc = bacc.Bacc("TRN2", target_bir_lowering=False, debug=False)

    embs_t = nc.dram_tensor("embs_s", [GPC, NG, DIM], f32, kind="ExternalInput")
    gabA_t = nc.dram_tensor("gabA", [128, 2, DIM], f32, kind="ExternalInput")
    gabB_t = nc.dram_tensor("gabB", [96, 2, DIM], f32, kind="ExternalInput")
    consts_t = nc.dram_tensor("consts", [128, 81], f32, kind="ExternalInput")
    out_t = nc.dram_tensor("out", [PPC, DIM + 1], f32, kind="ExternalOutput")
    out7_t = nc.dram_tensor("out7", [1, DIM], f32, kind="ExternalOutput")

    with tile.TileContext(nc) as tc:
        with (
            tc.tile_pool(name="pool", bufs=1) as pool,
            tc.tile_pool(name="psum", bufs=1, space=bass.MemorySpace.PSUM) as psum,
        ):
            # gather tile: cols (a0, b0, a1, b1); pad = partitions 96..127
            # of (a1, b1), memset to 1.0 (fake 8th pair -> asq=bsq=K, finite)
            gab = pool.tile([128, 4, DIM], f32, tag="gab")
            nc.gpsimd.memset(gab[96:128, 2:4, :], 1.0)

            # consts on the ACT HWDGE queue (parallel to the SP stream):
            #   cols 8g..8g+8: selector S_g (ones col 8g+g -> output row g)
            #   cols 64..71  : neg t0 block-ones (pair m -> rows 32m..32m+32)
            #   cols 72..79  : neg t1 block-ones (pairs 4..6 + pad block col 79)
            #   col 80       : all-ones (group-7 single-column selector)
            consts = pool.tile([128, 81], f32r, tag="consts")
            nc.scalar.dma_start(consts[:], consts_t.ap().bitcast(f32r))

            # SP stream, consumption order: gather rows then embs chunks;
            # the last chunk split into d-halves
            nc.sync.dma_start(gab[:, 0:2, :], gabA_t.ap())
            nc.sync.dma_start(gab[0:96, 2:4, :], gabB_t.ap())
            etiles = []
            for g in range(GPC):
                e = pool.tile([128, 2, DIM], f32r, tag=f"e{g}")
                src = embs_t.ap()[g].rearrange("(p h) d -> p h d", h=2).bitcast(f32r)
                if g < GPC - 1:
                    nc.sync.dma_start(e[:], src)
                else:
                    # row-halves (contiguous 2KB/partition descriptors): the
                    # group-7 matmuls consume each half straight from the DMA
                    nc.sync.dma_start(e[:, 0, :], src[:, 0, :])
                    nc.sync.dma_start(e[:, 1, :], src[:, 1, :])
                etiles.append(e)

            # --- negative path: a*b + tmp/sim on GpSimd, squares on the
            # DVE's idle window before the first embs chunk lands ---
            prods = []
            with nc.allow_low_precision(reason="f32r is fp32-width"):
                for t in range(2):
                    a, b = gab[:, 2 * t, :], gab[:, 2 * t + 1, :]
                    pr = pool.tile([128, DIM], f32r, tag=f"prod{t}")
                    aa = pool.tile([128, DIM], f32r, tag=f"aa{t}")
                    bb = pool.tile([128, DIM], f32r, tag=f"bb{t}")
                    nc.gpsimd.tensor_mul(pr[:], a, b)
                    nc.vector.tensor_mul(aa[:], a, a)
                    nc.vector.tensor_mul(bb[:], b, b)
                    prods.append((pr, aa, bb))

            # block-selector matmuls: K-sums per pair -> [8, 512] PSUM
            dot_ps = psum.tile([8, DIM], f32, tag="dot")
            asq_ps = psum.tile([8, DIM], f32, tag="asq")
            bsq_ps = psum.tile([8, DIM], f32, tag="bsq")
            for t, (pr, aa, bb) in enumerate(prods):
                blk = consts[:, 64 + 8 * t:72 + 8 * t]
                st, sp = (t == 0), (t == 1)
                nc.tensor.matmul(dot_ps[:], blk, pr[:], start=st, stop=sp)
                nc.tensor.matmul(asq_ps[:], blk, aa[:], start=st, stop=sp)
                nc.tensor.matmul(bsq_ps[:], blk, bb[:], start=st, stop=sp)

            # --- group sums: h-add on DVE, selector matmul accumulation.
            # Groups 0..6 -> one [8,512] chain (stop at g6); group 7 gets a
            # single-column chain into [1,512] (partition 0) so its copy
            # and output DMA stay legal and tiny on the tail.
            sums_ps = psum.tile([8, DIM], f32, tag="sums")
            s7_ps = psum.tile([1, DIM], f32, tag="s7")
            ctiles = [
                pool.tile([128, DIM], f32r, tag=f"c{g}", name=f"c{g}")
                for g in range(GPC - 1)
            ]

            out_sb = pool.tile([GPC, DIM + 1], f32, tag="out_sb")
            out7_sb = pool.tile([1, DIM], f32, tag="out7_sb")
            dot_sb = pool.tile([8, DIM], f32, tag="dot_sb")
            ai = pool.tile([8, DIM], f32, tag="ai")
            bi = pool.tile([8, DIM], f32, tag="bi")
            tmp = pool.tile([8, DIM], f32, tag="tmp")
            sim = pool.tile([8, DIM], f32, tag="sim")
            etile = pool.tile([8, DIM], f32, tag="etile")

            def hadd(g):
                with nc.allow_low_precision(reason="f32r is fp32-width"):
                    nc.vector.tensor_add(
                        ctiles[g][:], etiles[g][:, 0, :], etiles[g][:, 1, :]
                    )

            def selmm(g):
                nc.tensor.matmul(
                    sums_ps[:],
                    consts[:, 8 * g:8 * g + 8],
                    ctiles[g][:],
                    start=(g == 0),
                    stop=(g == GPC - 2),
                )

            # sim = dot * rsqrt(asq) * rsqrt(bsq)  (cosine; the reference eps
            # guard can never bind for randn inputs). The whole negative-path
            # epilogue runs on ACT + GpSimd: the DVE stream stays pure
            # stream-gated h-adds so the scheduler can't stall it on the
            # (late) ai/bi products.
            nc.scalar.copy(dot_sb[:], dot_ps[:])
            nc.scalar.activation(ai[:], asq_ps[:], AF.Abs_reciprocal_sqrt)
            nc.scalar.activation(bi[:], bsq_ps[:], AF.Abs_reciprocal_sqrt)
            nc.gpsimd.tensor_mul(tmp[:], dot_sb[:], ai[:])
            nc.gpsimd.tensor_mul(sim[:], tmp[:], bi[:])
            # exp(sim/T), row-sum fused straight into the out tile's den col
            nc.scalar.activation(
                etile[:], sim[:], AF.Exp,
                scale=float(1.0 / TEMP), accum_out=out_sb[:, DIM:DIM + 1],
            )
            for g in range(GPC - 1):
                hadd(g)
                selmm(g)
            nc.scalar.copy(out_sb[0:PPC, 0:DIM], sums_ps[0:PPC, :])
            nc.sync.dma_start(out_t.ap(), out_sb[0:PPC, :])

            g = GPC - 1
            nc.tensor.matmul(
                s7_ps[:], consts[:, 80:81], etiles[g][:, 0, :], start=True, stop=False
            )
            nc.tensor.matmul(
                s7_ps[:], consts[:, 80:81], etiles[g][:, 1, :], start=False, stop=True
            )
            nc.scalar.copy(out7_sb[:], s7_ps[:])
            nc.sync.dma_start(out7_t.ap(), out7_sb[:])

    nc.compile()
    return nc


def _get_program():
    global _PROGRAM
    if _PROGRAM is None:
        _PROGRAM = _build_program()
    return _PROGRAM


def kernel(embs, g0, g1, g2, neg1, neg2, **_unused):
    global LAST_RESULTS
    from concourse.bass_utils import run_bass_kernel_spmd

    embs = np.ascontiguousarray(np.asarray(embs, dtype=np.float32))
    g1 = np.ascontiguousarray(np.asarray(g1, dtype=np.float32))
    g2 = np.ascontiguousarray(np.asarray(g2, dtype=np.float32))
    neg1 = np.asarray(neg1).astype(np.int64)
    neg2 = np.asarray(neg2).astype(np.int64)

    consts = np.zeros((128, 81), np.float32)
    for g in range(GPC):
        consts[:, 8 * g + g] = 1.0                      # selector S_g, column g
    for m in range(4):
        consts[m * 32:(m + 1) * 32, 64 + m] = 1.0       # t0 pair blocks
    for j in range(3):
        consts[j * 32:(j + 1) * 32, 72 + 4 + j] = 1.0   # t1 pair blocks 4..6
    consts[96:128, 79] = 1.0                            # fake pad pair
    consts[:, 80] = 1.0                                 # group-7 ones column

    in_maps = []
    for c in range(N_CORES):
        idx1 = neg1[c * PPC:(c + 1) * PPC].reshape(-1)
        idx2 = neg2[c * PPC:(c + 1) * PPC].reshape(-1)
        gabA = np.empty((128, 2, DIM), np.float32)
        gabA[:, 0, :] = g1[idx1[:128]]
        gabA[:, 1, :] = g2[idx2[:128]]
        gabB = np.empty((96, 2, DIM), np.float32)
        gabB[:, 0, :] = g1[idx1[128:]]
        gabB[:, 1, :] = g2[idx2[128:]]
        in_maps.append({
            "embs_s": embs[c * GPC:(c + 1) * GPC],
            "gabA": gabA,
            "gabB": gabB,
            "consts": consts,
        })

    nc = _get_program()
    res = run_bass_kernel_spmd(nc, in_maps, core_ids=list(range(N_CORES)))
    LAST_RESULTS = res

    sums = np.empty((D, DIM), np.float64)
    den_neg = np.empty((P,), np.float64)
    for c in range(N_CORES):
        o = res.results[c]["out"]
        sums[c * GPC:c * GPC + PPC] = o[:, :DIM]
        sums[c * GPC + GPC - 1] = res.results[c]["out7"][0]
        den_neg[c * PPC:(c + 1) * PPC] = o[:, DIM]

    s_i, s_j = sums[:P], sums[L:]
    na = np.maximum(np.sqrt((s_i * s_i).sum(1)), EPS)
    nb = np.maximum(np.sqrt((s_j * s_j).sum(1)), EPS)
    pos = (s_i * s_j).sum(1) / (na * nb)
    num = np.exp(pos / TEMP)
    den = num + den_neg
    total = 2.0 * np.sum(np.log(den) - pos / TEMP)
    return np.asarray(total, dtype=np.float32)
